# revision 15
# baseline (speedup 1.0000x reference)
"""MoE layer (top-2 of 8 experts) for 8 Trainium2 NeuronCores.

Strategy: expert-parallel. Host computes the (tiny) router + top-2 dispatch in
numpy; core e runs expert e's FFN over its dispatched tokens (padded to a fixed
capacity C=2048 = the exact mean load) with bf16 matmuls (fp32 PSUM
accumulate); host scales by the combine probabilities and sums the two expert
outputs per token (legal: y*p = (silu(g)*u) @ dwT * p is linear in the down
matmul output).

The kernel is PE-bound: 1536 matmuls x 512 moving cols per core = 786432 PE
columns at 1 col/cycle (measured ~100% tensor busy in both the burst clock
regime and the P0 power-throttled regime, so wall time = PE cols / clock).
bf16 is chosen over fp32r: identical cols/cycle but half the DMA bytes and
lower PE power draw (measurably less P0 downclocking under sustained load),
at max-rel error 4e-3 vs the 2e-2 gate.

Per rep (VERSION=2): single pass over all weights. All 64 h tiles (4 token
tiles x 16 h-blocks, bf16, 8 MB) stay SBUF-resident, so gate/up weights
stream once (8 MB) and the down weights are one SBUF-resident tile (4 MB).
  stage A: gate^T/up^T [H, Ct] = gwT/uwT.T @ xt  (contract D, 8 k-tiles),
           h = silu(gate) * up  (ACT silu -> DVE mul, cast bf16)
  stage B: y[128d, 512tok] per (ct, dblock) = dw-block.T @ h  (contract H,
           16 k-tiles, 8 PSUM banks round-robin), ACT copy-cast to bf16 y

DMA on trn2 costs ~2.8us fixed per instruction, so the host pre-packs weights
and tokens into SBUF-tile order: every load is a multi-MB DMA with >=16KB
contiguous lines. Total traffic 20 MB/rep (x 4 + gw/uw 8 + dw 4 + y 4).
"""

import numpy as np

import concourse.bass as bass
import concourse.mybir as mybir
import concourse.tile as tile
from concourse import bacc
from concourse.bass_utils import run_bass_kernel_spmd

E = 8
TOP_K = 2
B, S, D, H = 4, 2048, 1024, 2048
T = B * S
C = 2048          # per-expert token capacity; overflow pairs (seed-0: ~137
                  # of 16384, counts max 2175) fall back to exact host compute
CT = 512          # token tile
P = 128
NCT = C // CT     # 5
KD = D // P       # 8  k-tiles for gate/up
KH = H // P       # 16 k-tiles for down
NH4 = H // 512    # 4  groups of 4 h-blocks
F32 = mybir.dt.float32
F32R = mybir.dt.float32r
BF16 = mybir.dt.bfloat16
AF = mybir.ActivationFunctionType

# matmul operand dtype: f32r (4B, ~2x PE power) or bf16 (2B). Same PE
# cols/cycle either way; bf16 halves DMA and PE power (less P0 throttle).
DT_MM = BF16
ND = D // P       # 8  d-blocks for the v2 down matmul
VERSION = 2       # 1 = two weight passes, prob-scale on device, f32 y
                  # 2 = single weight pass, host prob-scale, bf16 y


def emit_expert_ffn(tc, xt, gw, uw, dw, pv, y):
    """Emit one expert's FFN.

    DRAM tensors (all pre-packed on host):
      xt [NCT, 128, KD, 512] f32r - tokens, transposed per ct tile
      gw/uw [NH4, 128, KD, 512] f32r - gate/up weights per 4-h-block group
      dw [2, 2, 128, KH//2, 512] f32r - down weights per (dc, kh-half)
      pv [128, C//128] f32 - combine probs (token-partition layout)
      y  [NCT, 2, 128, 4, 512] f32 out - [ct, dc, p, m, 512]
    """
    nc = tc.nc
    # superblocks of up to 2 token tiles sharing one weight pass
    sbs = [list(range(s, min(s + 2, NCT))) for s in range(0, NCT, 2)]

    with (
        tc.tile_pool(name="xpool", bufs=2) as xpool,
        tc.tile_pool(name="wpool", bufs=3) as wpool,
        tc.tile_pool(name="hpool", bufs=36) as hpool,
        tc.tile_pool(name="dpool", bufs=3) as dpool,
        tc.tile_pool(name="tpool", bufs=5) as tpool,
        tc.tile_pool(name="opool", bufs=2) as opool,
        tc.tile_pool(name="ppool", bufs=1) as ppool,
        tc.tile_pool(name="pspool", bufs=8, space="PSUM") as pspool,
    ):
        p_sb = ppool.tile([P, C // P], F32)
        nc.gpsimd.dma_start(p_sb[:, :], pv[:, :])

        for cts in sbs:
            # ---- token tiles: one 2MB DMA per ct ----
            xts = []
            for ct in cts:
                x_t = xpool.tile([P, KD, CT], DT_MM, name=f"xt_{ct}", tag="xt")
                nc.gpsimd.dma_start(x_t[:, 0:4, :], xt[ct][:, 0:4, :])
                nc.gpsimd.dma_start(x_t[:, 4:8, :], xt[ct][:, 4:8, :])
                xts.append(x_t)
            hs = [
                [
                    hpool.tile([P, CT], DT_MM, name=f"h_{ct}_{ht}", tag="h")
                    for ht in range(KH)
                ]
                for ct in cts
            ]

            # ---- stage A: gate/up matmuls + silu*mul -> h ----
            first_sb = cts[0] == 0
            for ht4 in range(NH4):
                gt = wpool.tile([P, KD, 512], DT_MM, name=f"g_{ht4}", tag="w")
                if ht4 == 0 and first_sb:
                    # quarter-granularity on the very first load so the first
                    # matmuls start ~2us earlier out of the cold start
                    for q in range(4):
                        nc.sync.dma_start(
                            gt[:, 2 * q:2 * q + 2, :], gw[ht4][:, 2 * q:2 * q + 2, :]
                        )
                else:
                    nc.sync.dma_start(gt[:, 0:4, :], gw[ht4][:, 0:4, :])
                    nc.sync.dma_start(gt[:, 4:8, :], gw[ht4][:, 4:8, :])
                ut = wpool.tile([P, KD, 512], DT_MM, name=f"u_{ht4}", tag="w")
                nc.scalar.dma_start(ut[:, 0:4, :], uw[ht4][:, 0:4, :])
                nc.scalar.dma_start(ut[:, 4:8, :], uw[ht4][:, 4:8, :])
                # ct-major, all-gate-then-all-up: gt's last use lands at ~75%
                # of the group so the next group's weight DMA overlaps compute
                for ci in range(len(cts)):
                    tmps = []
                    for sub in range(4):
                        ht = ht4 * 4 + sub
                        pg = pspool.tile([P, CT], F32, name=f"pg_{ht}_{ci}", tag="ps")
                        for kt in range(KD):
                            nc.tensor.matmul(
                                pg[:, :],
                                gt[:, kt, sub * P:(sub + 1) * P],
                                xts[ci][:, kt, :],
                                start=(kt == 0),
                                stop=(kt == KD - 1),
                            )
                        tmp = tpool.tile([P, CT], F32, name=f"t_{ht}_{ci}", tag="t")
                        nc.scalar.activation(tmp[:, :], pg[:, :], AF.Silu)
                        tmps.append(tmp)
                    for sub in range(4):
                        ht = ht4 * 4 + sub
                        pu = pspool.tile([P, CT], F32, name=f"pu_{ht}_{ci}", tag="ps")
                        for kt in range(KD):
                            nc.tensor.matmul(
                                pu[:, :],
                                ut[:, kt, sub * P:(sub + 1) * P],
                                xts[ci][:, kt, :],
                                start=(kt == 0),
                                stop=(kt == KD - 1),
                            )
                        nc.vector.tensor_mul(
                            hs[ci][ht][:, :], tmps[sub][:, :], pu[:, :]
                        )

            # ---- stage B: down matmuls + prob scale -> y ----
            for dc in range(2):
                pos = {}
                for ci in range(len(cts)):
                    for m in range(CT // P):
                        pos[(ci, m)] = pspool.tile(
                            [P, 512], F32, name=f"po_{dc}_{ci}_{m}", tag="ps"
                        )
                ots = [
                    opool.tile([P, CT // P, 512], F32, name=f"o_{dc}_{ci}", tag="o")
                    for ci in range(len(cts))
                ]
                for hf in range(4):
                    dt_ = dpool.tile([P, KH // 4, 512], DT_MM, name=f"d_{dc}_{hf}", tag="dw")
                    nc.sync.dma_start(
                        dt_[:, :, :], dw[dc, hf // 2][:, (hf % 2) * 4:(hf % 2) * 4 + 4, :]
                    )
                    for kb in range(KH // 4):
                        kh = hf * (KH // 4) + kb
                        for ci in range(len(cts)):
                            for m in range(CT // P):
                                nc.tensor.matmul(
                                    pos[(ci, m)][:, :],
                                    hs[ci][kh][:, m * P:(m + 1) * P],
                                    dt_[:, kb, :],
                                    start=(kh == 0),
                                    stop=(kh == KH - 1),
                                )
                for ci, ct in enumerate(cts):
                    for m in range(CT // P):
                        j = ct * (CT // P) + m
                        nc.scalar.mul(
                            ots[ci][:, m, :], pos[(ci, m)][:, :], p_sb[:, j:j + 1]
                        )
                        # per-m stores start as soon as each eviction lands,
                        # shortening the kernel-tail drain
                        nc.gpsimd.dma_start(y[ct, dc][:, m, :], ots[ci][:, m, :])


def emit_expert_ffn_v2(tc, xt, gw, uw, dw, y):
    """Single-weight-pass variant (2-byte matmul dtype only).

    All 4 token tiles' h activations stay SBUF-resident in bf16 (8 MB), so
    gate/up weights stream exactly once per rep (8 MB) and the down weights
    are a single SBUF-resident tile (4 MB, one load). The down matmul flips
    operands vs v1: stationary = 128x128 dw blocks, moving = h tiles, output
    [128 d, 512 tok] per (ct, dblock). Outputs are UNSCALED expert FFN
    results in bf16; the host applies the combine probabilities (legal since
    y_scaled = (silu(g) * u) @ dwT * p is linear in the down matmul output).

    DRAM tensors:
      xt [NCT, 128, KD, 512]  - tokens, transposed per ct tile
      gw/uw [NH4, 128, KD, 512]
      dw [128, KH, ND, 128]   - [h-part within kh, kh, dblock, d]
      y  [NCT, ND, 128, 512]  - [ct, dblock, d-part, tok]
    """
    nc = tc.nc
    cts = list(range(NCT))
    with (
        tc.tile_pool(name="xpool", bufs=5) as xpool,
        tc.tile_pool(name="wpool", bufs=4) as wpool,
        tc.tile_pool(name="hpool", bufs=64) as hpool,
        tc.tile_pool(name="dpool", bufs=1) as dpool,
        tc.tile_pool(name="tpool", bufs=5) as tpool,
        tc.tile_pool(name="opool", bufs=8) as opool,
        tc.tile_pool(name="pspool", bufs=8, space="PSUM") as pspool,
    ):
        # Token loads get the gpsimd queue EXCLUSIVELY and are emitted first:
        # DMA queues are FIFO, and the next rep's first matmul needs x[0] -
        # anything queued ahead of it (the 4MB dwsb reload, which must wait
        # for the previous rep's last down matmul) would stall the PE at
        # every rep boundary.
        xts = []
        for ct in cts:
            x_t = xpool.tile([P, KD, CT], DT_MM, name=f"xt_{ct}", tag="xt")
            nc.gpsimd.dma_start(x_t[:, 0:4, :], xt[ct][:, 0:4, :])
            nc.gpsimd.dma_start(x_t[:, 4:8, :], xt[ct][:, 4:8, :])
            xts.append(x_t)

        # down weights: one resident tile, 32KB/partition (DMA emitted after
        # stage A's weight loads so the sync queue serves gw first)
        dwsb = dpool.tile([P, KH, ND, P], DT_MM)
        hs = [
            [
                hpool.tile([P, CT], DT_MM, name=f"h_{ct}_{ht}", tag="h")
                for ht in range(KH)
            ]
            for ct in cts
        ]

        # ---- stage A: gate/up matmuls + silu*mul -> h (single weight pass)
        for ht4 in range(NH4):
            gt = wpool.tile([P, KD, 512], DT_MM, name=f"g_{ht4}", tag="w")
            if ht4 == 0:
                for q in range(4):
                    nc.sync.dma_start(
                        gt[:, 2 * q:2 * q + 2, :], gw[ht4][:, 2 * q:2 * q + 2, :]
                    )
            else:
                nc.sync.dma_start(gt[:, 0:4, :], gw[ht4][:, 0:4, :])
                nc.sync.dma_start(gt[:, 4:8, :], gw[ht4][:, 4:8, :])
            ut = wpool.tile([P, KD, 512], DT_MM, name=f"u_{ht4}", tag="w")
            nc.scalar.dma_start(ut[:, 0:4, :], uw[ht4][:, 0:4, :])
            nc.scalar.dma_start(ut[:, 4:8, :], uw[ht4][:, 4:8, :])
            for ci in range(NCT):
                tmps = []
                for sub in range(4):
                    ht = ht4 * 4 + sub
                    pg = pspool.tile([P, CT], F32, name=f"pg_{ht}_{ci}", tag="ps")
                    for kt in range(KD):
                        nc.tensor.matmul(
                            pg[:, :],
                            gt[:, kt, sub * P:(sub + 1) * P],
                            xts[ci][:, kt, :],
                            start=(kt == 0),
                            stop=(kt == KD - 1),
                        )
                    tmp = tpool.tile([P, CT], F32, name=f"t_{ht}_{ci}", tag="t")
                    nc.scalar.activation(tmp[:, :], pg[:, :], AF.Silu)
                    tmps.append(tmp)
                for sub in range(4):
                    ht = ht4 * 4 + sub
                    pu = pspool.tile([P, CT], F32, name=f"pu_{ht}_{ci}", tag="ps")
                    for kt in range(KD):
                        nc.tensor.matmul(
                            pu[:, :],
                            ut[:, kt, sub * P:(sub + 1) * P],
                            xts[ci][:, kt, :],
                            start=(kt == 0),
                            stop=(kt == KD - 1),
                        )
                    nc.vector.tensor_mul(
                        hs[ci][ht][:, :], tmps[sub][:, :], pu[:, :]
                    )

        # down-weight load, on sync behind the 4 gw groups: its WAR wait (on
        # the previous rep's last down matmul) is long satisfied by the time
        # the queue reaches it, and the next rep's gw0 then has all of stage
        # B to transfer - no rep-boundary stall on either tensor
        for q in range(4):
            nc.sync.dma_start(
                dwsb[:, 4 * q:4 * q + 4, :, :], dw[:, 4 * q:4 * q + 4, :, :]
            )

        # ---- stage B: down matmuls, stationary = dw blocks, moving = h ----
        for ci, ct in enumerate(cts):
            pos = [
                pspool.tile([P, CT], F32, name=f"po_{ct}_{db}", tag="ps")
                for db in range(ND)
            ]
            for kh in range(KH):
                for db in range(ND):
                    nc.tensor.matmul(
                        pos[db][:, :],
                        dwsb[:, kh, db, :],
                        hs[ci][kh][:, :],
                        start=(kh == 0),
                        stop=(kh == KH - 1),
                    )
            for db in range(ND):
                o = opool.tile([P, CT], DT_MM, name=f"o_{ct}_{db}", tag="o")
                nc.scalar.copy(o[:, :], pos[db][:, :])
                # y stores on scalar - keeps gpsimd free for x prefetch; the
                # next rep's uw0 queues behind only ~1MB of tail stores
                nc.scalar.dma_start(y[ct, db][:, :], o[:, :])


def build_nc(reps_loop=False, max_reps=512, version=None):
    """Build the per-core Bass program. With reps_loop, the whole body runs
    inside a For_i whose trip count is read from an int32 input "reps"."""
    if version is None:
        version = VERSION
    nc = bacc.Bacc(None, target_bir_lowering=False)
    with tile.TileContext(nc) as tc:
        xt = nc.dram_tensor("xt", [NCT, P, KD, CT], DT_MM, kind="ExternalInput")
        gw = nc.dram_tensor("gw", [NH4, P, KD, 512], DT_MM, kind="ExternalInput")
        uw = nc.dram_tensor("uw", [NH4, P, KD, 512], DT_MM, kind="ExternalInput")
        if version == 2:
            dw = nc.dram_tensor("dw", [P, KH, ND, P], DT_MM, kind="ExternalInput")
            y = nc.dram_tensor("y", [NCT, ND, P, CT], DT_MM, kind="ExternalOutput")
            body = lambda: emit_expert_ffn_v2(tc, xt, gw, uw, dw, y)
        else:
            dw = nc.dram_tensor("dw", [2, 2, P, KH // 2, 512], DT_MM, kind="ExternalInput")
            pv = nc.dram_tensor("pv", [P, C // P], F32, kind="ExternalInput")
            y = nc.dram_tensor("y", [NCT, 2, P, CT // P, 512], F32, kind="ExternalOutput")
            body = lambda: emit_expert_ffn(tc, xt, gw, uw, dw, pv, y)
        if reps_loop:
            reps = nc.dram_tensor("reps", [1, 1], mybir.dt.int32, kind="ExternalInput")
            with tc.tile_pool(name="rpool", bufs=1) as rpool:
                r_sb = rpool.tile([1, 1], mybir.dt.int32)
                nc.sync.dma_start(r_sb[:, :], reps[:, :])
                rv = nc.values_load(
                    r_sb[0:1, 0:1],
                    min_val=0,
                    max_val=max_reps,
                    skip_runtime_bounds_check=True,
                )
            with tc.For_i(0, rv, 1):
                body()
        else:
            body()
    nc.compile()
    return nc


def pack_inputs(x_pad, gate_w_e, up_w_e, down_w_e, p_pad):
    """Pack one expert's inputs into the SBUF-tile-order DRAM layouts."""
    # xt [NCT, 128, KD, 512]: [ct, p, kt, tok] = x_pad[ct*512+tok, kt*128+p]
    xt = np.ascontiguousarray(
        x_pad.reshape(NCT, CT, KD, P).transpose(0, 3, 2, 1)
    )
    # gw/uw [NH4, 128, KD, 512]: [b, p, kt, h] = w[b*512+h, kt*128+p]
    gw = np.ascontiguousarray(
        gate_w_e.reshape(NH4, 512, KD, P).transpose(0, 3, 2, 1)
    )
    uw = np.ascontiguousarray(
        up_w_e.reshape(NH4, 512, KD, P).transpose(0, 3, 2, 1)
    )
    # dw [2, 2, 128, KH//2, 512]: [dc, hf, p, kb, d] = down[dc*512+d, hf*1024+kb*128+p]
    dw = np.ascontiguousarray(
        down_w_e.reshape(2, 512, 2, KH // 2, P).transpose(0, 2, 4, 3, 1)
    )
    pv = np.ascontiguousarray(p_pad.reshape(C // P, P).T)
    dtnp = mybir.dt.np(DT_MM)
    return {"xt": xt.astype(dtnp), "gw": gw.astype(dtnp),
            "uw": uw.astype(dtnp), "dw": dw.astype(dtnp), "pv": pv}


def unpack_y(y_pack):
    """y_pack [NCT, 2, 128, 4, 512] -> y [C, D]."""
    return np.ascontiguousarray(
        y_pack.transpose(0, 3, 2, 1, 4).reshape(C, D)
    )


def pack_inputs_v2(x_pad, gate_w_e, up_w_e, down_w_e):
    """Pack one expert's inputs for the v2 layouts (no pv; dw d-blocked)."""
    xt = np.ascontiguousarray(
        x_pad.reshape(NCT, CT, KD, P).transpose(0, 3, 2, 1)
    )
    gw = np.ascontiguousarray(
        gate_w_e.reshape(NH4, 512, KD, P).transpose(0, 3, 2, 1)
    )
    uw = np.ascontiguousarray(
        up_w_e.reshape(NH4, 512, KD, P).transpose(0, 3, 2, 1)
    )
    # dw [128, KH, ND, 128]: [p, kh, db, j] = down[db*128+j, kh*128+p]
    dw = np.ascontiguousarray(
        down_w_e.reshape(ND, P, KH, P).transpose(3, 2, 0, 1)
    )
    dtnp = mybir.dt.np(DT_MM)
    return {"xt": xt.astype(dtnp), "gw": gw.astype(dtnp),
            "uw": uw.astype(dtnp), "dw": dw.astype(dtnp)}


def unpack_y_v2(y_pack):
    """y_pack [NCT, ND, 128, 512] (bf16, unscaled) -> y [C, D] f32."""
    return np.ascontiguousarray(
        y_pack.astype(np.float32).transpose(0, 3, 1, 2).reshape(C, D)
    )


def prepare_in_maps(inputs):
    """Route tokens and build the per-core input maps for VERSION. Returns
    (in_maps, route) where route = (ee, tt, pp, pos, counts, starts, order)."""
    x = np.ascontiguousarray(
        np.asarray(inputs["hidden_states"], np.float32).reshape(T, D)
    )
    router_w = np.asarray(inputs["router_w"], np.float32)
    gate_w = np.asarray(inputs["gate_w"], np.float32)
    up_w = np.asarray(inputs["up_w"], np.float32)
    down_w = np.asarray(inputs["down_w"], np.float32)
    route = route_and_dispatch(x, router_w)
    ee, tt, pp, pos, counts, starts, order = route
    in_maps = []
    for e in range(E):
        n_e = min(int(counts[e]), C)
        sel = order[starts[e]:starts[e] + n_e]   # pairs dispatched to core e
        xp = np.zeros((C, D), np.float32)
        xp[:n_e] = x[tt[sel]]
        if VERSION == 2:
            in_maps.append(pack_inputs_v2(xp, gate_w[e], up_w[e], down_w[e]))
        else:
            pvec = np.zeros(C, np.float32)
            pvec[:n_e] = pp[sel]
            in_maps.append(
                pack_inputs(xp, gate_w[e], up_w[e], down_w[e], pvec)
            )
    return in_maps, route


def route_and_dispatch(x, router_w):
    """Host router + top-2 dispatch (matches softmax/top_k/renorm of the
    reference exactly)."""
    logits = x @ router_w.T                      # [T, E]
    t_ar = np.arange(T)
    i1 = np.argmax(logits, axis=1)
    l1 = logits[t_ar, i1]
    lm = logits.copy()
    lm[t_ar, i1] = -np.inf
    i2 = np.argmax(lm, axis=1)
    l2 = lm[t_ar, i2]
    e2 = np.exp(l2 - l1)
    p1 = 1.0 / (1.0 + e2)
    p2 = e2 / (1.0 + e2)

    ee = np.concatenate([i1, i2])                # [2T] expert of each pair
    tt = np.concatenate([t_ar, t_ar])            # [2T] token of each pair
    pp = np.concatenate([p1, p2]).astype(np.float32)
    counts = np.bincount(ee, minlength=E)
    starts = np.zeros(E, np.int64)
    starts[1:] = np.cumsum(counts)[:-1]
    order = np.argsort(ee, kind="stable")
    pos = np.empty(2 * T, np.int64)
    pos[order] = np.arange(2 * T) - starts[ee[order]]
    return ee, tt, pp, pos, counts, starts, order


def kernel(**inputs):
    x = np.ascontiguousarray(
        np.asarray(inputs["hidden_states"], np.float32).reshape(T, D)
    )
    gate_w = np.asarray(inputs["gate_w"], np.float32)
    up_w = np.asarray(inputs["up_w"], np.float32)
    down_w = np.asarray(inputs["down_w"], np.float32)

    in_maps, route = prepare_in_maps(inputs)
    ee, tt, pp, pos, counts, starts, order = route

    nc = build_nc()
    res = run_bass_kernel_spmd(nc, in_maps, core_ids=list(range(E)))
    unpack = unpack_y_v2 if VERSION == 2 else unpack_y
    ys = np.stack(
        [unpack(res.results[e]["y"]) for e in range(E)]
    ).reshape(E * C, D)

    ok = pos < C
    contrib = np.zeros((2 * T, D), np.float32)
    g = ee * C + pos
    if VERSION == 2:
        # v2 returns unscaled expert outputs; apply combine probs on host
        contrib[ok] = ys[g[ok]] * pp[ok, None]
    else:
        contrib[ok] = ys[g[ok]]
    # capacity-overflow fallback: exact fp32 host compute for the few pairs
    # beyond capacity (~0.8% of pairs for the seed-0 routing), batched per
    # expert
    if not ok.all():
        bad = np.nonzero(~ok)[0]
        for e in np.unique(ee[bad]):
            js = bad[ee[bad] == e]
            xb = x[tt[js]]
            gb = xb @ gate_w[e].T
            ub = xb @ up_w[e].T
            hb = (gb / (1.0 + np.exp(-gb))) * ub
            contrib[js] = (hb @ down_w[e].T) * pp[js, None]
    out = contrib[:T] + contrib[T:]
    return out.reshape(B, S, D).astype(np.float32)

